# revision 11
# baseline (speedup 1.0000x reference)
"""Trainium2 Bass kernel for nn_Conv2d_20590073217670.

Conv2d: input [32,64,64,64] (NCHW), weight [576,128] (unfold layout:
row = ci*9 + a*3 + b for tap (a,b)), bias [1,128,1,1], stride 1, pad 1.
Output [32,128,64,64].

Strategy: data-parallel over batch — 4 images per NeuronCore, 8 cores.
Per image, implicit GEMM in bf16 with two padded [128, 66, 66] SBUF
layouts, all built by direct fp32->bf16 casts from the staged input
(DVE casts the lower halves, Act the upper halves, in parallel):
  xb: parts 0:64  = img[c, r-1, j-1]   (zero border all sides)
      parts 64:128 = img[c, r,   j-1]  (one row up)
  xc: parts 0:64  = img[c, r-1, j-1]
      parts 64:128 = img[c, r-1, j  ]  (one col left)
Per 8-row output block, 5 matmul passes, each a uniform full [8, 64]
PSUM tile (border taps read zero padding):
  3x K=128: vertical tap pairs (0,b)+(1,b) from xb        (b = 0,1,2)
  1x K=128: horizontal tap pair (2,0)+(2,1) from xc at +2 rows
  1x K=64 : tap (2,2) from xb lower at +2 rows, col 2
Four blocks are processed pass-major so consecutive matmuls rotate
over 4 PSUM banks and pipeline through the PE array (same-bank
accumulation serializes at ~465ns/matmul; rotated it runs at ~220ns).
PSUM eviction (fused bias add) alternates DVE/Act per bank; output
streams out per 16 rows alternating between the two HWDGE rings.
Inputs are chunked in three so the first matmul can start early.
"""
import sys

for _p in ("/opt/trn_rl_repo", "/root/.axon_site/_ro/trn_rl_repo"):
    if _p not in sys.path:
        sys.path.append(_p)

import numpy as np
from contextlib import ExitStack

import concourse.bacc as bacc
import concourse.tile as tile
from concourse import mybir
from concourse.bass_utils import run_bass_kernel_spmd

f32 = mybir.dt.float32
bf16 = mybir.dt.bfloat16

N_CORES = 8
NB = 4  # images per core


def build_nc():
    nc = bacc.Bacc()
    x = nc.declare_dram_parameter("x", [NB, 64, 64, 64], f32, isOutput=False)
    w = nc.declare_dram_parameter("w", [576, 128], f32, isOutput=False)
    bias = nc.declare_dram_parameter("b", [128, 1], f32, isOutput=False)
    out = nc.declare_dram_parameter("out", [NB, 128, 64, 64], f32, isOutput=True)

    with tile.TileContext(nc) as tc, ExitStack() as ctx:
        const = ctx.enter_context(tc.tile_pool(name="const", bufs=1))
        xf_pool = ctx.enter_context(tc.tile_pool(name="xf", bufs=2))
        ob_pool = ctx.enter_context(tc.tile_pool(name="ob", bufs=2))
        ps_pool = ctx.enter_context(tc.tile_pool(name="ps", bufs=2, space="PSUM"))

        # ---- weights.  wb [128, 9, 128] bf16: partition p<64 holds channel
        # p's taps 0..8; partition 64+ci holds taps 3..8 at slots 0..5, so
        # wb[:, b, :] pairs taps (0,b) lower / (1,b) upper and wb[0:64, 8, :]
        # is tap (2,2).  wc [128, 128] pairs taps (2,0) lower / (2,1) upper.
        w3 = w[:].rearrange("(c t) m -> c t m", t=9)
        ws = const.tile([128, 9, 128], f32)
        wsc = const.tile([128, 128], f32)
        wb = const.tile([128, 9, 128], bf16)
        wc = const.tile([128, 128], bf16)
        bt = const.tile([128, 1], f32)
        nc.scalar.dma_start(out=ws[0:64, :, :], in_=w3)
        nc.scalar.dma_start(out=ws[64:128, 0:6, :], in_=w3[:, 3:9, :])
        nc.scalar.dma_start(out=wsc[0:64, :], in_=w3[:, 6, :])
        nc.scalar.dma_start(out=wsc[64:128, :], in_=w3[:, 7, :])
        nc.scalar.dma_start(out=bt[:], in_=bias[:])
        nc.vector.tensor_copy(wb[0:64, :, :], ws[0:64, :, :])
        nc.vector.tensor_copy(wb[64:128, 0:6, :], ws[64:128, 0:6, :])
        nc.vector.tensor_copy(wc[:, :], wsc[:, :])

        # ---- two persistent padded image tile sets, manually double-
        # buffered; interiors are rewritten every image, borders zeroed once
        xb0 = const.tile([128, 66, 66], bf16)
        xb1 = const.tile([128, 66, 66], bf16)
        xc0 = const.tile([128, 66, 66], bf16)
        xc1 = const.tile([128, 66, 66], bf16)
        for xb in (xb0, xb1):
            nc.gpsimd.memset(xb[0:64, 0:1, :], 0.0)
            nc.gpsimd.memset(xb[0:64, 65:66, :], 0.0)
            nc.gpsimd.memset(xb[0:64, :, 0:1], 0.0)
            nc.gpsimd.memset(xb[0:64, :, 65:66], 0.0)
            nc.gpsimd.memset(xb[64:128, 0:64, 0:1], 0.0)
            nc.gpsimd.memset(xb[64:128, 0:64, 65:66], 0.0)
        for xc in (xc0, xc1):
            nc.gpsimd.memset(xc[0:64, 65:66, :], 0.0)
            nc.gpsimd.memset(xc[0:64, :, 0:1], 0.0)
            nc.gpsimd.memset(xc[64:128, 65:66, 0:64], 0.0)

        # input/cast chunk boundaries (image rows); chunks A+B cover the
        # first 4-block half's reads (padded rows <= 34), C the rest
        CH = ((0, 18), (18, 34), (34, 64))

        for n in range(NB):
            xf = xf_pool.tile([64, 64, 64], f32)
            for r0, r1 in CH:
                nc.sync.dma_start(out=xf[:, r0:r1, :], in_=x[n][:, r0:r1, :])

            xb = (xb0, xb1)[n % 2]
            xc = (xc0, xc1)[n % 2]
            for r0, r1 in CH:
                # two casts in parallel (DVE lower, Act upper); xc's halves
                # are bf16 copies of the cast rows on the idle SWDGE queue
                nc.vector.tensor_copy(xb[0:64, 1 + r0:1 + r1, 1:65], xf[:, r0:r1, :])
                nc.scalar.copy(xb[64:128, r0:r1, 1:65], xf[:, r0:r1, :])
                nc.gpsimd.dma_start(
                    out=xc[0:64, 1 + r0:1 + r1, :], in_=xb[0:64, 1 + r0:1 + r1, :])
                nc.gpsimd.dma_start(
                    out=xc[64:128, 1 + r0:1 + r1, 0:65],
                    in_=xb[0:64, 1 + r0:1 + r1, 1:66])

            osb = ob_pool.tile([128, 64, 64], f32)
            for half in range(2):
                # pass-major over 4 blocks: consecutive matmuls rotate over 4
                # PSUM banks, pipelining the PE and reusing each weight 4x
                P0 = ps_pool.tile([128, 8, 64], f32)
                P1 = ps_pool.tile([128, 8, 64], f32)
                P2 = ps_pool.tile([128, 8, 64], f32)
                P3 = ps_pool.tile([128, 8, 64], f32)
                Ps = (P0, P1, P2, P3)
                ys = [half * 32 + q * 8 for q in range(4)]
                for p in range(5):
                    st, sp = (p == 0), (p == 4)
                    for P, y0 in zip(Ps, ys):
                        if p < 3:
                            nc.tensor.matmul(
                                P[:, :, :], wb[:, p, :],
                                xb[:, y0:y0 + 8, p:p + 64],
                                start=st, stop=sp,
                            )
                        elif p == 3:
                            nc.tensor.matmul(
                                P[:, :, :], wc[:, :],
                                xc[:, y0 + 2:y0 + 10, 0:64],
                                start=st, stop=sp,
                            )
                        else:
                            nc.tensor.matmul(
                                P[:, :, :], wb[0:64, 8, :],
                                xb[0:64, y0 + 2:y0 + 10, 2:66],
                                start=st, stop=sp,
                            )
                for q, (P, y0) in enumerate(zip(Ps, ys)):
                    if q != 1:
                        nc.vector.tensor_scalar_add(osb[:, y0:y0 + 8, :], P[:, :, :], bt[:])
                    else:
                        nc.scalar.add(osb[:, y0:y0 + 8, :], P[:, :, :], bt[:])
                    if n == NB - 1:
                        # last image: stream each 8-row bank out immediately
                        dma_eng = nc.scalar if q % 2 == 0 else nc.sync
                        dma_eng.dma_start(
                            out=out[n][:, y0:y0 + 8, :], in_=osb[:, y0:y0 + 8, :])
                    elif q % 2 == 1:
                        dma_eng = nc.scalar if (half * 2 + q // 2) % 2 == 0 else nc.sync
                        dma_eng.dma_start(
                            out=out[n][:, y0 - 8:y0 + 8, :],
                            in_=osb[:, y0 - 8:y0 + 8, :])

    nc.finalize()
    return nc


_NC = None


def _get_nc():
    global _NC
    if _NC is None:
        _NC = build_nc()
    return _NC


def kernel(**inputs) -> np.ndarray:
    x = np.ascontiguousarray(np.asarray(inputs["input"], dtype=np.float32))
    w = np.ascontiguousarray(np.asarray(inputs["weight"], dtype=np.float32))
    b = np.ascontiguousarray(
        np.asarray(inputs["bias"], dtype=np.float32).reshape(128, 1))
    nc = _get_nc()
    in_maps = [
        {"x": x[c * NB:(c + 1) * NB], "w": w, "b": b} for c in range(N_CORES)
    ]
    res = run_bass_kernel_spmd(nc, in_maps, list(range(N_CORES)))
    return np.concatenate([r["out"] for r in res.results], axis=0)


# revision 15
# speedup vs baseline: 1.2435x; 1.2435x over previous
"""Trainium2 Bass kernel for nn_Conv2d_20590073217670.

Conv2d: input [32,64,64,64] (NCHW), weight [576,128] (unfold layout:
row = ci*9 + a*3 + b for tap (a,b)), bias [1,128,1,1], stride 1, pad 1.
Output [32,128,64,64].

Strategy: data-parallel over batch — 4 images per NeuronCore, 8 cores.
Per image, implicit GEMM in bf16 with two padded [128, 66, 66] SBUF
layouts, all built by direct fp32->bf16 casts from the staged input
(DVE casts the lower halves, Act the upper halves, in parallel):
  xb: parts 0:64  = img[c, r-1, j-1]   (zero border all sides)
      parts 64:128 = img[c, r,   j-1]  (one row up)
  xc: parts 0:64  = img[c, r-1, j-1]
      parts 64:128 = img[c, r-1, j  ]  (one col left)
Per 8-row output block, 5 matmul passes, each a uniform full [8, 64]
PSUM tile (border taps read zero padding):
  3x K=128: vertical tap pairs (0,b)+(1,b) from xb        (b = 0,1,2)
  1x K=128: horizontal tap pair (2,0)+(2,1) from xc at +2 rows
  1x K=64 : tap (2,2) from xb lower at +2 rows, col 2
Four blocks are processed pass-major so consecutive matmuls rotate
over 4 PSUM banks and pipeline through the PE array (same-bank
accumulation serializes at ~465ns/matmul; rotated it runs at ~220ns).
PSUM eviction (fused bias add) alternates DVE/Act per bank; output
streams out per 16 rows alternating between the two HWDGE rings.
Inputs are chunked in three so the first matmul can start early.
"""
import sys

for _p in ("/opt/trn_rl_repo", "/root/.axon_site/_ro/trn_rl_repo"):
    if _p not in sys.path:
        sys.path.append(_p)

import numpy as np
from contextlib import ExitStack

import concourse.bacc as bacc
import concourse.tile as tile
from concourse import mybir
from concourse.bass_utils import run_bass_kernel_spmd

f32 = mybir.dt.float32
bf16 = mybir.dt.bfloat16

N_CORES = 8
NB = 4  # images per core


def build_nc():
    nc = bacc.Bacc()
    x = nc.declare_dram_parameter("x", [NB, 64, 64, 64], f32, isOutput=False)
    w = nc.declare_dram_parameter("w", [576, 128], f32, isOutput=False)
    bias = nc.declare_dram_parameter("b", [128, 1], f32, isOutput=False)
    out = nc.declare_dram_parameter("out", [NB, 128, 64, 64], f32, isOutput=True)

    with tile.TileContext(nc) as tc, ExitStack() as ctx:
        const = ctx.enter_context(tc.tile_pool(name="const", bufs=1))
        xf_pool = ctx.enter_context(tc.tile_pool(name="xf", bufs=2))
        ob_pool = ctx.enter_context(tc.tile_pool(name="ob", bufs=2))
        ps_pool = ctx.enter_context(tc.tile_pool(name="ps", bufs=2, space="PSUM"))

        # ---- weights.  wb [128, 9, 128] bf16: partition p<64 holds channel
        # p's taps 0..8; partition 64+ci holds taps 3..8 at slots 0..5, so
        # wb[:, b, :] pairs taps (0,b) lower / (1,b) upper and wb[0:64, 8, :]
        # is tap (2,2).  wc [128, 128] pairs taps (2,0) lower / (2,1) upper.
        w3 = w[:].rearrange("(c t) m -> c t m", t=9)
        ws = const.tile([128, 9, 128], f32)
        wb = const.tile([128, 9, 128], bf16)
        bt = const.tile([128, 1], f32)
        nc.scalar.dma_start(out=ws[0:64, :, :], in_=w3)
        nc.scalar.dma_start(out=ws[64:128, 0:6, :], in_=w3[:, 3:9, :])
        nc.scalar.dma_start(out=bt[:], in_=bias[:])
        nc.vector.tensor_copy(wb[0:64, :, :], ws[0:64, :, :])
        nc.vector.tensor_copy(wb[64:128, 0:6, :], ws[64:128, 0:6, :])

        # ---- two persistent padded image tile sets, manually double-
        # buffered; interiors are rewritten every image, borders zeroed once
        xb0 = const.tile([128, 66, 66], bf16)
        xb1 = const.tile([128, 66, 66], bf16)
        for xb in (xb0, xb1):
            nc.gpsimd.memset(xb[0:64, 0:1, :], 0.0)
            nc.gpsimd.memset(xb[0:64, 65:66, :], 0.0)
            nc.gpsimd.memset(xb[0:64, :, 0:1], 0.0)
            nc.gpsimd.memset(xb[0:64, :, 65:66], 0.0)
            nc.gpsimd.memset(xb[64:128, 0:64, 0:1], 0.0)
            nc.gpsimd.memset(xb[64:128, 0:64, 65:66], 0.0)

        # input/cast chunk boundaries (image rows); chunks A+B cover the
        # first 4-block half's reads (padded rows <= 34), C the rest
        CH = ((0, 18), (18, 34), (34, 64))

        for n in range(NB):
            xf = xf_pool.tile([64, 64, 64], f32)
            for r0, r1 in CH:
                nc.sync.dma_start(out=xf[:, r0:r1, :], in_=x[n][:, r0:r1, :])

            xb = (xb0, xb1)[n % 2]
            for r0, r1 in CH:
                # two casts in parallel: DVE fills the lower half, Act the
                # row-shifted upper half, straight from the fp32 staging tile
                nc.vector.tensor_copy(xb[0:64, 1 + r0:1 + r1, 1:65], xf[:, r0:r1, :])
                nc.scalar.copy(xb[64:128, r0:r1, 1:65], xf[:, r0:r1, :])

            osb = ob_pool.tile([128, 64, 64], f32)
            for half in range(2):
                # pass-major over 4 blocks: consecutive matmuls rotate over 4
                # PSUM banks, pipelining the PE and reusing each weight 4x
                P0 = ps_pool.tile([128, 8, 64], f32)
                P1 = ps_pool.tile([128, 8, 64], f32)
                P2 = ps_pool.tile([128, 8, 64], f32)
                P3 = ps_pool.tile([128, 8, 64], f32)
                Ps = (P0, P1, P2, P3)
                ys = [half * 32 + q * 8 for q in range(4)]
                for p in range(6):
                    st, sp = (p == 0), (p == 5)
                    for P, y0 in zip(Ps, ys):
                        if p < 3:
                            nc.tensor.matmul(
                                P[:, :, :], wb[:, p, :],
                                xb[:, y0:y0 + 8, p:p + 64],
                                start=st, stop=sp,
                            )
                        else:
                            b = p - 3
                            nc.tensor.matmul(
                                P[:, :, :], wb[0:64, 6 + b, :],
                                xb[0:64, y0 + 2:y0 + 10, b:b + 64],
                                start=st, stop=sp,
                            )
                for q, (P, y0) in enumerate(zip(Ps, ys)):
                    if q != 1:
                        nc.vector.tensor_scalar_add(osb[:, y0:y0 + 8, :], P[:, :, :], bt[:])
                    else:
                        nc.scalar.add(osb[:, y0:y0 + 8, :], P[:, :, :], bt[:])
                    if n == NB - 1:
                        # last image: stream each 8-row bank out immediately
                        dma_eng = nc.scalar if q % 2 == 0 else nc.sync
                        dma_eng.dma_start(
                            out=out[n][:, y0:y0 + 8, :], in_=osb[:, y0:y0 + 8, :])
                    elif q % 2 == 1:
                        dma_eng = nc.scalar if (half * 2 + q // 2) % 2 == 0 else nc.sync
                        dma_eng.dma_start(
                            out=out[n][:, y0 - 8:y0 + 8, :],
                            in_=osb[:, y0 - 8:y0 + 8, :])

    nc.finalize()
    return nc


_NC = None


def _get_nc():
    global _NC
    if _NC is None:
        _NC = build_nc()
    return _NC


def kernel(**inputs) -> np.ndarray:
    x = np.ascontiguousarray(np.asarray(inputs["input"], dtype=np.float32))
    w = np.ascontiguousarray(np.asarray(inputs["weight"], dtype=np.float32))
    b = np.ascontiguousarray(
        np.asarray(inputs["bias"], dtype=np.float32).reshape(128, 1))
    nc = _get_nc()
    in_maps = [
        {"x": x[c * NB:(c + 1) * NB], "w": w, "b": b} for c in range(N_CORES)
    ]
    res = run_bass_kernel_spmd(nc, in_maps, list(range(N_CORES)))
    return np.concatenate([r["out"] for r in res.results], axis=0)


# revision 17
# speedup vs baseline: 1.2590x; 1.0124x over previous
"""Trainium2 Bass kernel for nn_Conv2d_20590073217670.

Conv2d: input [32,64,64,64] (NCHW), weight [576,128] (unfold layout:
row = ci*9 + a*3 + b for tap (a,b)), bias [1,128,1,1], stride 1, pad 1.
Output [32,128,64,64].

Strategy: data-parallel over batch — 4 images per NeuronCore, 8 cores.
Per image, implicit GEMM in bf16 with two padded [128, 66, 66] SBUF
layouts, all built by direct fp32->bf16 casts from the staged input
(DVE casts the lower halves, Act the upper halves, in parallel):
  xb: parts 0:64  = img[c, r-1, j-1]   (zero border all sides)
      parts 64:128 = img[c, r,   j-1]  (one row up)
  xc: parts 0:64  = img[c, r-1, j-1]
      parts 64:128 = img[c, r-1, j  ]  (one col left)
Per 8-row output block, 5 matmul passes, each a uniform full [8, 64]
PSUM tile (border taps read zero padding):
  3x K=128: vertical tap pairs (0,b)+(1,b) from xb        (b = 0,1,2)
  1x K=128: horizontal tap pair (2,0)+(2,1) from xc at +2 rows
  1x K=64 : tap (2,2) from xb lower at +2 rows, col 2
Four blocks are processed pass-major so consecutive matmuls rotate
over 4 PSUM banks and pipeline through the PE array (same-bank
accumulation serializes at ~465ns/matmul; rotated it runs at ~220ns).
PSUM eviction (fused bias add) alternates DVE/Act per bank; output
streams out per 16 rows alternating between the two HWDGE rings.
Inputs are chunked in three so the first matmul can start early.
"""
import sys

for _p in ("/opt/trn_rl_repo", "/root/.axon_site/_ro/trn_rl_repo"):
    if _p not in sys.path:
        sys.path.append(_p)

import numpy as np
from contextlib import ExitStack

import concourse.bacc as bacc
import concourse.tile as tile
from concourse import mybir
from concourse.bass_utils import run_bass_kernel_spmd

f32 = mybir.dt.float32
bf16 = mybir.dt.bfloat16

N_CORES = 8
NB = 4  # images per core


def build_nc():
    nc = bacc.Bacc()
    x = nc.declare_dram_parameter("x", [NB, 64, 64, 64], f32, isOutput=False)
    w = nc.declare_dram_parameter("w", [576, 128], f32, isOutput=False)
    bias = nc.declare_dram_parameter("b", [128, 1], f32, isOutput=False)
    out = nc.declare_dram_parameter("out", [NB, 128, 64, 64], f32, isOutput=True)

    with tile.TileContext(nc) as tc, ExitStack() as ctx:
        const = ctx.enter_context(tc.tile_pool(name="const", bufs=1))
        xf_pool = ctx.enter_context(tc.tile_pool(name="xf", bufs=2))
        ob_pool = ctx.enter_context(tc.tile_pool(name="ob", bufs=2))
        ps_pool = ctx.enter_context(tc.tile_pool(name="ps", bufs=2, space="PSUM"))

        # ---- weights.  wb [128, 9, 128] bf16: partition p<64 holds channel
        # p's taps 0..8; partition 64+ci holds taps 3..8 at slots 0..5, so
        # wb[:, b, :] pairs taps (0,b) lower / (1,b) upper and wb[0:64, 8, :]
        # is tap (2,2).  wc [128, 128] pairs taps (2,0) lower / (2,1) upper.
        w3 = w[:].rearrange("(c t) m -> c t m", t=9)
        ws = const.tile([128, 9, 128], f32)
        wb = const.tile([128, 9, 128], bf16)
        bt = const.tile([128, 1], f32)
        nc.scalar.dma_start(out=ws[0:64, :, :], in_=w3)
        nc.scalar.dma_start(out=ws[64:128, 0:6, :], in_=w3[:, 3:9, :])
        nc.scalar.dma_start(out=bt[:], in_=bias[:])
        nc.vector.tensor_copy(wb[0:64, :, :], ws[0:64, :, :])
        nc.vector.tensor_copy(wb[64:128, 0:6, :], ws[64:128, 0:6, :])

        # ---- two persistent padded image tile sets, manually double-
        # buffered; interiors are rewritten every image, borders zeroed once
        xb0 = const.tile([128, 66, 66], bf16)
        xb1 = const.tile([128, 66, 66], bf16)
        for xb in (xb0, xb1):
            nc.gpsimd.memset(xb[0:64, 0:1, :], 0.0)
            nc.gpsimd.memset(xb[0:64, 65:66, :], 0.0)
            nc.gpsimd.memset(xb[0:64, :, 0:1], 0.0)
            nc.gpsimd.memset(xb[0:64, :, 65:66], 0.0)
            nc.gpsimd.memset(xb[64:128, 0:64, 0:1], 0.0)
            nc.gpsimd.memset(xb[64:128, 0:64, 65:66], 0.0)

        # input/cast chunk boundaries (image rows); chunks A+B cover the
        # first 4-block half's reads (padded rows <= 34), C the rest
        CH = ((0, 18), (18, 34), (34, 64))

        for n in range(NB):
            # image 0 is on the critical path: chunk its input/casts so the
            # first matmuls start early; later images are prefetched whole
            chunks = CH if n == 0 else ((0, 64),)
            xf = xf_pool.tile([64, 64, 64], f32)
            for r0, r1 in chunks:
                nc.sync.dma_start(out=xf[:, r0:r1, :], in_=x[n][:, r0:r1, :])

            xb = (xb0, xb1)[n % 2]
            for r0, r1 in chunks:
                # two casts in parallel: DVE fills the lower half, Act the
                # row-shifted upper half, straight from the fp32 staging tile
                nc.vector.tensor_copy(xb[0:64, 1 + r0:1 + r1, 1:65], xf[:, r0:r1, :])
                nc.scalar.copy(xb[64:128, r0:r1, 1:65], xf[:, r0:r1, :])

            osb = ob_pool.tile([128, 64, 64], f32)
            for half in range(2):
                # pass-major over 4 blocks: consecutive matmuls rotate over 4
                # PSUM banks, pipelining the PE and reusing each weight 4x
                P0 = ps_pool.tile([128, 8, 64], f32)
                P1 = ps_pool.tile([128, 8, 64], f32)
                P2 = ps_pool.tile([128, 8, 64], f32)
                P3 = ps_pool.tile([128, 8, 64], f32)
                Ps = (P0, P1, P2, P3)
                ys = [half * 32 + q * 8 for q in range(4)]
                for p in range(6):
                    st, sp = (p == 0), (p == 5)
                    for P, y0 in zip(Ps, ys):
                        if p < 3:
                            nc.tensor.matmul(
                                P[:, :, :], wb[:, p, :],
                                xb[:, y0:y0 + 8, p:p + 64],
                                start=st, stop=sp,
                            )
                        else:
                            b = p - 3
                            nc.tensor.matmul(
                                P[:, :, :], wb[0:64, 6 + b, :],
                                xb[0:64, y0 + 2:y0 + 10, b:b + 64],
                                start=st, stop=sp,
                            )
                for q, (P, y0) in enumerate(zip(Ps, ys)):
                    if q != 1:
                        nc.vector.tensor_scalar_add(osb[:, y0:y0 + 8, :], P[:, :, :], bt[:])
                    else:
                        nc.scalar.add(osb[:, y0:y0 + 8, :], P[:, :, :], bt[:])
                    if q % 2 == 1:
                        dma_eng = nc.scalar if (half * 2 + q // 2) % 2 == 0 else nc.sync
                        dma_eng.dma_start(
                            out=out[n][:, y0 - 8:y0 + 8, :],
                            in_=osb[:, y0 - 8:y0 + 8, :])

    nc.finalize()
    return nc


_NC = None


def _get_nc():
    global _NC
    if _NC is None:
        _NC = build_nc()
    return _NC


def kernel(**inputs) -> np.ndarray:
    x = np.ascontiguousarray(np.asarray(inputs["input"], dtype=np.float32))
    w = np.ascontiguousarray(np.asarray(inputs["weight"], dtype=np.float32))
    b = np.ascontiguousarray(
        np.asarray(inputs["bias"], dtype=np.float32).reshape(128, 1))
    nc = _get_nc()
    in_maps = [
        {"x": x[c * NB:(c + 1) * NB], "w": w, "b": b} for c in range(N_CORES)
    ]
    res = run_bass_kernel_spmd(nc, in_maps, list(range(N_CORES)))
    return np.concatenate([r["out"] for r in res.results], axis=0)
